# revision 1
# baseline (speedup 1.0000x reference)
"""Trainium2 Bass kernel for a single pre-norm transformer block — fp8 rewrite.

Reference (B=2, T=2048, C=768, H=12, HD=64):
    x = x + causal_attn(LN1(x) @ W_qkv) @ W_attn_proj
    x = x + gelu(LN2(x) @ W_fc) @ W_mlp_proj

Sharding as v1: 8 cores, zero collectives, core c = (batch c//4, rank
p=c%4), interleaved q-blocks {15-p,11-p,7-p,3-p}, uniform
SLOT_BOUNDS=(16,12,8,4).  Design vs the 262us bf16 v1:

  * All GEMMs fp8e4m3 + MatmulPerfMode.DoubleRow (2x128 contraction at
    0.5 cyc/row = 4x bf16).  Scores contract the 64 head features as two
    32-partition slices (K/Q stored [32p, 2, t] per head, 4 heads per
    tile, explicit tile_position for the w=96 window).  W_fc/W_mlp_proj
    are hi+lo fp8 DR pairs against a stride-0-duplicated rhs — weight
    quantization error drops to ~0.1% while keeping the DR rate.
  * LN1 never materialized: QKV run on RAW fp8 x; per-token rstd rides
    the PSUM evacuation op; -mu (x) colsum(W) is one fp8-DR fixup
    matmul per output tile (host ships colsum(W)/8).
  * Causal masks ADDED into score PSUM by fp8 DR matmuls (maskT @ I)
    pre-exp: exp underflows masked lanes to exact fp8 zero; exp writes
    fp8 directly and AV runs DR on it (ones/64 column -> denominators).
  * Softmax 1/D: DVE reciprocal -> fp8, partition-broadcast by one fp8
    DR matmul; odd heads shift to partitions 64-127 via one tiny DMA.
  * LN stats: s1/s2 by fp8-DR ones-matmuls (x^2 on gpsimd via
    tensor_scalar |x|^2 in-line), fused scalar_tensor_tensor tails,
    token-major rstd via 16 one-column f32r matmuls.
  * Emission interleaves K/Q/V production with attention head groups so
    exp (the Act-engine bottleneck, ~65us) starts ~15us in and stays fed.
"""

import sys

if "/opt/trn_rl_repo" not in sys.path:
    sys.path.insert(0, "/opt/trn_rl_repo")

import numpy as np

import concourse.bass as bass
import concourse.mybir as mybir
from concourse import bacc
import concourse.tile as tile

P = 128
B, T, C, H, HD = 2, 2048, 768, 12, 64
OWN = 512
NF = C // P          # 6 feature chunks
NP3 = NF // 2        # 3 DR feature pairs
NQT = T // 512       # 4 key-column tiles
NTILE = NQT + 1      # + own-query tile
NKT = T // P         # 16 key chunks
NHC = 24             # h chunks in mlp
NH2 = 12             # h chunk pairs in mlp proj
SLOT_BOUNDS = (16, 12, 8, 4)
EPS = 1e-5
WS = 8.0             # host scale split: wsum/8 on host, *8 in B'
DEN = 32.0           # denominator pre-scale (V ones column = 1/DEN)

f32 = mybir.dt.float32
f32r = mybir.dt.float32r
bf16 = mybir.dt.bfloat16
fp8 = mybir.dt.float8e4
AFT = mybir.ActivationFunctionType
ALU = mybir.AluOpType
DR = mybir.MatmulPerfMode.DoubleRow

GELU_FUNC = AFT.Gelu  # dev sims patch an erf-gelu into bass_interp for this
MASK_NEG = -224.0


def _r(ap):
    return ap.bitcast(f32r)


def _dup2(ap, n):
    """[128, n] -> [128, 2, n] with stride-0 slice dim (shared DR rhs)."""
    return ap.unsqueeze(1).broadcast_to([P, 2, n])


def build_program(unit_gb=True, debug=False):
    nc = bacc.Bacc()
    d = {}

    def par(name, shape, dt, out=False):
        d[name] = nc.declare_dram_parameter(name, shape, dt, out)[:]

    par("x8", [P, NF, NTILE * 512], fp8)
    par("xo", [P, NF, OWN], f32)
    par("Wqk", [2, 3, P, 2, NP3, 2, P], fp8)   # (q/k, hg, p, half, fpair, sl, col)
    par("Wv", [2, P, NP3, 2, 384], fp8)        # (vg, p, fpair, sl, col)
    par("Wap", [P, NP3, 2, C], fp8)
    par("Wfc", [P, 12, NF, 2, 256], fp8)       # (p, gp, fchunk, hi/lo, 2x128)
    par("Wmp", [P, NF, NH2, 2, P], fp8)        # (p, mo, hcpair, slice, col)
    par("wsqk", [1, 2, 3, 2, 2, P], fp8)       # sl1=0
    par("wsv", [1, 2, 2, 384], fp8)            # sl1=0
    par("maskT", [P, 4, 4, 2, P], fp8)         # sl1=0
    par("ident", [P, 2, P], fp8)               # sl0=I, sl1=0
    par("outT", [C, OWN], f32, out=True)
    if debug:
        par("dbg_x2", [P, NF, 512], f32, out=True)
        par("dbg_xln2", [P, NF, 2, 512], fp8, out=True)
        par("dbg_h", [P, NHC, 512], fp8, out=True)
        par("dbg_y", [P, NF, OWN], fp8, out=True)
    if not unit_gb:
        par("g2c", [P, NF], f32)
        par("b2c", [P, NF], f32)
    outT_r = d["outT"].rearrange("(o p) q -> p o q", p=P)

    with tile.TileContext(nc) as tc:
        _body(nc, tc, unit_gb, d, outT_r, debug)
    nc.finalize()
    return nc


def _body(nc, tc, unit_gb, d, outT_r, debug=False):
    from contextlib import ExitStack

    with ExitStack() as ctx:
        def pool(name, bufs, space="SBUF"):
            return ctx.enter_context(tc.tile_pool(name=name, bufs=bufs, space=space))

        singles = pool("singles", 1)

        # ---------------- constants ----------------
        ones8 = singles.tile([P, 2, P], fp8)
        nc.vector.memset(ones8[:], 1.0)
        onesf = singles.tile([P, HD], f32)
        nc.vector.memset(onesf[:], 1.0)
        onesq = singles.tile([P, 2, HD], fp8)
        nc.vector.memset(onesq[:], 1.0 / DEN)
        one1 = singles.tile([P, 512], f32)
        nc.vector.memset(one1[:], 1.0)
        eps_sb = singles.tile([P, 1], f32)
        nc.vector.memset(eps_sb[:], EPS)

        ident = singles.tile([P, 2, P], fp8)
        nc.sync.dma_start(out=ident[:], in_=d["ident"])
        maskT = singles.tile([P, 4, 4, 2, P], fp8)
        wsqk = singles.tile([1, 2, 3, 2, 2, P], fp8)
        nc.sync.dma_start(out=wsqk[:], in_=d["wsqk"])
        wsv = singles.tile([1, 2, 2, 384], fp8)
        nc.sync.dma_start(out=wsv[:], in_=d["wsv"])
        if not unit_gb:
            g2c = singles.tile([P, NF], f32)
            nc.sync.dma_start(out=g2c[:], in_=d["g2c"])
            b2c = singles.tile([P, NF], f32)
            nc.sync.dma_start(out=b2c[:], in_=d["b2c"])

        # LN1 products (live through attention prep)
        statp = pool("statp", 1)
        R_t = [statp.tile([P, 512], f32, name=f"R{qt}") for qt in range(NTILE)]
        Bp8 = statp.tile([P, NTILE, 2, 512], fp8, name="Bp8")
        nc.vector.memset(Bp8[:, :, 1, :], 0.0)
        rtok = statp.tile([P, NKT], f32, name="rtok")
        recp8 = statp.tile([P, 2, 512], fp8, name="recp8")
        nc.vector.memset(recp8[:, 1, :], 0.0)
        neg8c = statp.tile([P, 512], f32, name="neg8c")
        nc.vector.memset(neg8c[:], -WS / C)

        xpool = pool("xpool", NTILE)
        sqpool = pool("sqpool", 2)
        tailp = pool("tailp", 2)

        x8t = [None] * NTILE
        for qt in (0, NQT):
            t = xpool.tile([P, NF, 512], fp8, tag="x8", name=f"x8_{qt}")
            nc.sync.dma_start(out=t[:], in_=d["x8"][:, :, qt * 512:(qt + 1) * 512])
            x8t[qt] = t

        # ============ shared PSUM pools: stats -> attention ============
        ctxB = ExitStack()
        ps_mm = ctxB.enter_context(tc.tile_pool(name="ps_mm", bufs=2, space="PSUM"))
        ps_s = ctxB.enter_context(tc.tile_pool(name="ps_s", bufs=2, space="PSUM"))
        ps_y = ctxB.enter_context(tc.tile_pool(name="ps_y", bufs=2, space="PSUM"))
        wqk_p = pool("wqk_p", 2)
        wv_p = pool("wv_p", 2)
        vpool = pool("vpool", 1)
        apool = pool("apool", 3)
        ypool = pool("ypool", 1)
        ytp = pool("ytp", 2)
        recp = pool("recp", 2)
        kqpool = pool("kqpool", 1)

        # weights for head-group 0 queue right behind x8_0
        wqk_tiles = {}
        for qk in (1, 0):
            w_ = wqk_p.tile([P, 2, NP3, 2, P], fp8, tag="wqk", name=f"wqk{qk}_0")
            nc.sync.dma_start(out=w_[:], in_=d["Wqk"][qk, 0])
            wqk_tiles[(qk, 0)] = w_
        wv_tiles = {}
        wv0 = wv_p.tile([P, NP3, 2, 384], fp8, tag="wv", name="wv0")
        nc.sync.dma_start(out=wv0[:], in_=d["Wv"][0])
        wv_tiles[0] = wv0
        for qt in range(1, NQT):
            t = xpool.tile([P, NF, 512], fp8, tag="x8", name=f"x8_{qt}")
            nc.sync.dma_start(out=t[:], in_=d["x8"][:, :, qt * 512:(qt + 1) * 512])
            x8t[qt] = t
        nc.sync.dma_start(out=maskT[:], in_=d["maskT"])

        v_sb = vpool.tile([P, NKT, H, 96], fp8)
        nc.gpsimd.memset(v_sb[:, :, :, HD:96], 0.0)
        nc.gpsimd.memset(v_sb[:, :, :, HD:HD + 1], 1.0 / DEN)
        y_sb = ypool.tile([P, NF, OWN], fp8)
        rt_ps = ps_y.tile([P, NKT], f32, tag="y", name="rt_ps")

        def emit_stats(qt):
            sq = sqpool.tile([P, NF, 512], fp8, tag="sq", name=f"sq{qt}")
            for f in range(NF):
                eng = nc.vector if f % 3 else nc.gpsimd
                eng.tensor_tensor(sq[:, f, :], x8t[qt][:, f, :],
                                  x8t[qt][:, f, :], ALU.mult)
            st = ps_s.tile([P, 2, 512], f32, tag="s", name=f"st{qt}")
            for fp_ in range(NP3):
                nc.tensor.matmul(st[:, 0, :], ones8[:],
                                 x8t[qt][:, 2 * fp_:2 * fp_ + 2, :],
                                 start=(fp_ == 0), stop=(fp_ == NP3 - 1),
                                 perf_mode=DR)
                nc.tensor.matmul(st[:, 1, :], ones8[:],
                                 sq[:, 2 * fp_:2 * fp_ + 2, :],
                                 start=(fp_ == 0), stop=(fp_ == NP3 - 1),
                                 perf_mode=DR)
            s1 = st[:, 0, :]
            s2 = st[:, 1, :]
            mu = tailp.tile([P, 512], f32, tag="tl", name=f"mu{qt}")
            nc.vector.tensor_scalar(mu[:], s1, 1.0 / C, None, ALU.mult)
            u = tailp.tile([P, 512], f32, tag="tl", name=f"u{qt}")
            nc.vector.tensor_tensor(u[:], mu[:], mu[:], ALU.mult)
            v_ = tailp.tile([P, 512], f32, tag="tl", name=f"v{qt}")
            nc.vector.tensor_scalar(v_[:], s2, 1.0 / C, None, ALU.mult)
            nc.vector.tensor_tensor(v_[:], v_[:], u[:], ALU.subtract)
            sd = tailp.tile([P, 512], f32, tag="tl", name=f"sd{qt}")
            nc.scalar.activation(out=sd[:], in_=v_[:], func=AFT.Sqrt,
                                 bias=eps_sb[:])
            nc.vector.reciprocal(R_t[qt][:], sd[:])
            nc.vector.tensor_tensor(Bp8[:, qt, 0, :], s1, neg8c[:], ALU.mult)
            if qt < NQT:
                for j in range(4):
                    kt = 4 * qt + j
                    nc.tensor.matmul(rt_ps[:, kt:kt + 1],
                                     R_t[qt][0:1, j * P:(j + 1) * P],
                                     onesf[0:1, 0:1], start=True, stop=True)
                nc.vector.tensor_copy(out=rtok[:, 4 * qt:4 * qt + 4],
                                      in_=rt_ps[:, 4 * qt:4 * qt + 4])

        def emit_kq_tile(qk, hg, kt_sb, ti):
            w_ = wqk_tiles[(qk, hg)]
            qt = ti if qk == 1 else NQT
            for hf in range(2):
                ps = ps_mm.tile([P, 512], f32, tag="mm", name="kqps")
                for fp_ in range(NP3):
                    nc.tensor.matmul(ps[:], w_[:, hf, fp_, :, :],
                                     x8t[qt][:, 2 * fp_:2 * fp_ + 2, :],
                                     start=(fp_ == 0), stop=False,
                                     perf_mode=DR)
                nc.tensor.matmul(ps[:], wsqk[0:1, qk, hg, hf, :, :],
                                 Bp8[0:1, qt, :, :], start=False, stop=True,
                                 perf_mode=DR)
                nc.vector.tensor_tensor(
                    kt_sb[:, hf, ti * 512:(ti + 1) * 512],
                    ps[:], R_t[qt][:], ALU.mult)

        def emit_v_chunk(vg, kt):
            wv_ = wv_tiles[vg]
            qt, j = divmod(kt, 4)
            ps = ps_mm.tile([P, 384], f32, tag="mm", name="vps")
            for fp_ in range(NP3):
                nc.tensor.matmul(
                    ps[:], x8t[qt][:, 2 * fp_:2 * fp_ + 2, j * P:(j + 1) * P],
                    wv_[:, fp_, :, :], start=(fp_ == 0), stop=False,
                    perf_mode=DR)
            nc.tensor.matmul(ps[:], Bp8[0:1, qt, :, j * P:(j + 1) * P],
                             wsv[0:1, vg, :, :], start=False,
                             stop=True, perf_mode=DR)
            nc.vector.tensor_scalar(
                v_sb[:, kt, 6 * vg:6 * vg + 6, 0:HD],
                ps[:].rearrange("p (h e) -> p h e", e=HD),
                rtok[:, kt:kt + 1], None, ALU.mult)

        def emit_head_slot(h, k_sb, q_sb, s, yp):
            if yp is None:
                yp = ps_y.tile([P, 512], f32, tag="y", name=f"yp{h}")
            w0 = 32 * (h % 4)
            BS = SLOT_BOUNDS[s]
            groups = [8] * (BS // 8) + ([BS % 8] if BS % 8 else [])
            k0 = 0
            for gi, gs in enumerate(groups):
                sp = ps_s.tile([P, 8, P], f32, tag="s", name=f"sp{h}_{s}_{gi}")
                for jj in range(gs):
                    kt = k0 + jj
                    masked = kt >= BS - 4
                    nc.tensor.matmul(
                        sp[:, jj, :],
                        k_sb[w0:w0 + 32, :, kt * P:(kt + 1) * P],
                        q_sb[w0:w0 + 32, :, s * P:(s + 1) * P],
                        start=True, stop=not masked, perf_mode=DR,
                        tile_position=(w0, 0))
                    if masked:
                        nc.tensor.matmul(
                            sp[:, jj, :], maskT[:, s, kt - (BS - 4), :, :],
                            ident[:], start=False, stop=True, perf_mode=DR)
                a_sb = apool.tile([P, 8, P], fp8, tag="a",
                                  name=f"a{h}_{s}_{gi}")
                nc.scalar.activation(out=a_sb[:, :gs, :], in_=sp[:, :gs, :],
                                     func=AFT.Exp,
                                     scale=float(1.0 / np.sqrt(HD)))
                for u2 in range(gs // 2):
                    kt = k0 + 2 * u2
                    nc.tensor.matmul(
                        yp[0:96, s * P:(s + 1) * P],
                        v_sb[:, kt:kt + 2, h, 0:96],
                        a_sb[:, 2 * u2:2 * u2 + 2, :],
                        start=(kt == 0), stop=(kt == BS - 2), perf_mode=DR)
                k0 += gs
            return yp

        def emit_head_tail(h, yp):
            rec = recp.tile([P, 512], f32, tag="rec", name=f"rec{h}")
            nc.vector.reciprocal(rec[HD:HD + 1, :], yp[HD:HD + 1, :])
            nc.vector.tensor_tensor(recp8[HD:HD + 1, 0, :], rec[HD:HD + 1, :],
                                    one1[HD:HD + 1, :], ALU.mult)
            bc = ps_y.tile([HD, 512], f32, tag="y", name=f"bc{h}")
            nc.tensor.matmul(bc[:], onesq[HD:HD + 1, :, :],
                             recp8[HD:HD + 1, :, :], start=True, stop=True,
                             perf_mode=DR)
            bcs = recp.tile([HD, 512], f32, tag="bcs", name=f"bcs{h}")
            nc.vector.tensor_copy(out=bcs[:], in_=bc[:])
            if h % 2 == 0:
                nc.vector.tensor_tensor(y_sb[0:HD, h // 2, :], yp[0:HD, :],
                                        bcs[:], ALU.mult)
            else:
                yt = ytp.tile([HD, 512], fp8, tag="yt", name=f"yt{h}")
                nc.vector.tensor_tensor(yt[:], yp[0:HD, :], bcs[:], ALU.mult)
                nc.gpsimd.dma_start(out=y_sb[HD:P, h // 2, :], in_=yt[:])

        # interleaved emission: per-tile stats -> K(hg0) -> V(vg0) chunks,
        # with head 0's slots woven in so Act starts immediately
        kq_tiles = {}
        kq_tiles[(1, 0)] = kqpool.tile([P, 2, NQT * 512], fp8, name="kq1_0")
        kq_tiles[(0, 0)] = kqpool.tile([P, 2, 512], fp8, name="kq0_0")
        for qt in range(NQT):
            emit_stats(qt)
            emit_kq_tile(1, 0, kq_tiles[(1, 0)], qt)
            for kt in range(4 * qt, 4 * qt + 4):
                emit_v_chunk(0, kt)
            if qt == 0:
                emit_stats(NQT)
                emit_kq_tile(0, 0, kq_tiles[(0, 0)], 0)
        for hg in range(3):
            if hg > 0:
                for qk in (1, 0):
                    w_ = wqk_p.tile([P, 2, NP3, 2, P], fp8, tag="wqk",
                                    name=f"wqk{qk}_{hg}")
                    nc.sync.dma_start(out=w_[:], in_=d["Wqk"][qk, hg])
                    wqk_tiles[(qk, hg)] = w_
                if hg == 1:
                    wv1 = wv_p.tile([P, NP3, 2, 384], fp8, tag="wv", name="wv1")
                    nc.sync.dma_start(out=wv1[:], in_=d["Wv"][1])
                    wv_tiles[1] = wv1
                kq_tiles[(1, hg)] = kqpool.tile([P, 2, NQT * 512], fp8,
                                                name=f"kq1_{hg}")
                for ti in range(NQT):
                    emit_kq_tile(1, hg, kq_tiles[(1, hg)], ti)
                kq_tiles[(0, hg)] = kqpool.tile([P, 2, 512], fp8,
                                                name=f"kq0_{hg}")
                emit_kq_tile(0, hg, kq_tiles[(0, hg)], 0)
                if hg == 1:
                    for kt in range(NKT):
                        emit_v_chunk(1, kt)
            for hi in range(4):
                h = 4 * hg + hi
                yp = None
                for s in (3, 2, 1, 0):
                    yp = emit_head_slot(h, kq_tiles[(1, hg)],
                                        kq_tiles[(0, hg)], s, yp)
                emit_head_tail(h, yp)
        ctxB.close()

        # ============ attn proj + residual + LN2 ============
        ctxC = ExitStack()
        ps_mm2 = ctxC.enter_context(tc.tile_pool(name="ps_mm2", bufs=3, space="PSUM"))
        ps_st2 = ctxC.enter_context(tc.tile_pool(name="ps_st2", bufs=1, space="PSUM"))
        wap_sb = singles.tile([P, NP3, 2, C], fp8)
        nc.sync.dma_start(out=wap_sb[:], in_=d["Wap"])
        xo_sb = singles.tile([P, NF, OWN], f32)
        nc.gpsimd.dma_start(out=xo_sb[:], in_=d["xo"])
        x2pool = pool("x2pool", 1)
        x2 = x2pool.tile([P, NF, 512], f32)
        x28 = x2pool.tile([P, NF, 512], fp8)
        sq2 = x2pool.tile([P, NF, 512], fp8)
        for mo in range(NF):
            ps = ps_mm2.tile([P, 512], f32, tag="mm2", name="aps")
            for r2 in range(NP3):
                nc.tensor.matmul(ps[:], wap_sb[:, r2, :, mo * P:(mo + 1) * P],
                                 y_sb[:, 2 * r2:2 * r2 + 2, :],
                                 start=(r2 == 0), stop=(r2 == NP3 - 1),
                                 perf_mode=DR)
            nc.vector.tensor_tensor(x2[:, mo, :], ps[:], xo_sb[:, mo, :],
                                    ALU.add)
            nc.gpsimd.tensor_copy(out=x28[:, mo, :], in_=x2[:, mo, :])
            nc.vector.tensor_tensor(sq2[:, mo, :], x2[:, mo, :], x2[:, mo, :],
                                    ALU.mult)

        if debug:
            nc.sync.dma_start(out=d["dbg_x2"], in_=x2[:])
            nc.sync.dma_start(out=d["dbg_y"], in_=y_sb[:])
        st2 = ps_st2.tile([P, 2, 512], f32, name="st2")
        for fp_ in range(NP3):
            nc.tensor.matmul(st2[:, 0, :], ones8[:],
                             x28[:, 2 * fp_:2 * fp_ + 2, :],
                             start=(fp_ == 0), stop=(fp_ == NP3 - 1), perf_mode=DR)
            nc.tensor.matmul(st2[:, 1, :], ones8[:],
                             sq2[:, 2 * fp_:2 * fp_ + 2, :],
                             start=(fp_ == 0), stop=(fp_ == NP3 - 1), perf_mode=DR)
        s1b, s2b = st2[:, 0, :], st2[:, 1, :]
        R2 = singles.tile([P, 512], f32)
        MU2 = singles.tile([P, 512], f32)
        nc.vector.tensor_scalar(MU2[:], s1b, 1.0 / C, None, ALU.mult)
        u2_ = tailp.tile([P, 512], f32, tag="tl", name="u_ln2")
        nc.vector.tensor_tensor(u2_[:], MU2[:], MU2[:], ALU.mult)
        v2_ = tailp.tile([P, 512], f32, tag="tl", name="v_ln2")
        nc.vector.tensor_scalar(v2_[:], s2b, 1.0 / C, None, ALU.mult)
        nc.vector.tensor_tensor(v2_[:], v2_[:], u2_[:], ALU.subtract)
        sd2 = tailp.tile([P, 512], f32, tag="tl", name="sd_ln2")
        nc.scalar.activation(out=sd2[:], in_=v2_[:], func=AFT.Sqrt,
                             bias=eps_sb[:])
        nc.vector.reciprocal(R2[:], sd2[:])
        R2_16 = singles.tile([P, 512], f32)
        nc.vector.tensor_scalar(R2_16[:], R2[:], 1.0 / 16.0, None, ALU.mult)

        xln2 = singles.tile([P, NF, 2, 512], fp8)
        for f in range(NF):
            e1 = nc.vector if f < 3 else nc.gpsimd
            t_ = tailp.tile([P, 512], f32, tag="tl", name=f"xm{f}")
            e1.tensor_tensor(t_[:], x2[:, f, :], MU2[:], ALU.subtract)
            if not unit_gb:
                tg_ = tailp.tile([P, 512], f32, tag="tl", name=f"xg{f}")
                nc.vector.scalar_tensor_tensor(tg_[:], t_[:], g2c[:, f:f + 1],
                                               R2[:], ALU.mult, ALU.mult)
                nc.vector.tensor_scalar(xln2[:, f, 0, :], tg_[:],
                                        b2c[:, f:f + 1], None, ALU.add)
            else:
                e1.tensor_tensor(xln2[:, f, 0, :], t_[:], R2[:], ALU.mult)
            e2 = nc.gpsimd if f < 3 else nc.vector
            e2.tensor_tensor(xln2[:, f, 1, :], t_[:], R2_16[:], ALU.mult)
        ctxC.close()

        if debug:
            nc.sync.dma_start(out=d["dbg_xln2"], in_=xln2[:])
        # ============ MLP (hi+lo fp8 weights) ============
        # FC/gelu per h-chunk; proj contraction split in half so its first
        # pass overlaps the second half of FC instead of waiting for all 24
        # gelu outputs.
        ctxD = ExitStack()
        ps_fc = ctxD.enter_context(tc.tile_pool(name="ps_fc", bufs=2, space="PSUM"))
        ps_pj = ctxD.enter_context(tc.tile_pool(name="ps_pj", bufs=6, space="PSUM"))
        wfc_p = pool("wfc_p", 2)
        wmp_p = pool("wmp_p", 6)
        hpool = pool("hpool", 1)
        opool = pool("opool", 2)
        h_sb = hpool.tile([P, NHC, 512], fp8)
        pj = [ps_pj.tile([P, 512], f32, tag="pj", name=f"pj{mo}")
              for mo in range(NF)]

        def emit_fc(gp):
            wfc_ = wfc_p.tile([P, NF, 2, 256], fp8, tag="wfc", name=f"wfc{gp}")
            nc.sync.dma_start(out=wfc_[:], in_=d["Wfc"][:, gp])
            for i in range(2):
                hc = 2 * gp + i
                psf = ps_fc.tile([P, 512], f32, tag="fc", name=f"fps{hc}")
                for f in range(NF):
                    nc.tensor.matmul(psf[:],
                                     wfc_[:, f, :, i * P:(i + 1) * P],
                                     xln2[:, f, :, :],
                                     start=(f == 0), stop=(f == NF - 1),
                                     perf_mode=DR)
                nc.scalar.activation(out=h_sb[:, hc, :], in_=psf[:],
                                     func=GELU_FUNC)

        def emit_proj(mo, lo, hi_, wmp_):
            for hp in range(lo, hi_):
                nc.tensor.matmul(pj[mo][:], wmp_[:, hp - lo, :, :],
                                 h_sb[:, 2 * hp:2 * hp + 2, :],
                                 start=(hp == 0), stop=(hp == NH2 - 1),
                                 perf_mode=DR)

        for gp in range(6):
            emit_fc(gp)
        wmp_a = []
        for mo in range(NF):
            w_ = wmp_p.tile([P, 6, 2, P], fp8, tag="wmp", name=f"wmpA{mo}")
            nc.gpsimd.dma_start(out=w_[:], in_=d["Wmp"][:, mo, 0:6])
            wmp_a.append(w_)
        wmp_b = []
        for gp in range(6, 12):
            emit_fc(gp)
            emit_proj(gp - 6, 0, 6, wmp_a[gp - 6])
            w_ = wmp_p.tile([P, 6, 2, P], fp8, tag="wmp", name=f"wmpB{gp - 6}")
            nc.gpsimd.dma_start(out=w_[:], in_=d["Wmp"][:, gp - 6, 6:12])
            wmp_b.append(w_)
        for mo in range(NF):
            emit_proj(mo, 6, NH2, wmp_b[mo])
            ot = opool.tile([P, 512], f32, tag="o", name=f"o{mo}")
            nc.vector.tensor_tensor(ot[:], pj[mo][:], x2[:, mo, :], ALU.add)
            nc.gpsimd.dma_start(out=outT_r[:, mo, :], in_=ot[:])
        ctxD.close()


# ---------------------------------------------------------------------------
# host side
# ---------------------------------------------------------------------------

def make_core_inputs(inputs):
    """Build the 8 per-core input maps (layout + dtype prep only)."""
    import ml_dtypes

    e4 = ml_dtypes.float8_e4m3

    x = np.asarray(inputs["x"], np.float32)
    g1 = np.asarray(inputs["ln1_g"], np.float32)
    if not np.all(np.asarray(inputs["ln1_b"]) == 0.0):
        raise NotImplementedError("nonzero ln1_b not supported")
    Wqkv = np.asarray(inputs["W_qkv"], np.float32) * g1[:, None]
    Wq = Wqkv[:, 0:C]
    Wk = Wqkv[:, C:2 * C]
    Wv = Wqkv[:, 2 * C:3 * C]
    Wap = np.asarray(inputs["W_attn_proj"], np.float32)
    Wfc = np.asarray(inputs["W_fc"], np.float32)
    Wmp = np.asarray(inputs["W_mlp_proj"], np.float32)

    def fchunk(a, ncol):
        # [C, ncol] -> [128, NF, ncol] with feature f = o*128 + p
        return np.ascontiguousarray(a.reshape(NF, P, ncol).transpose(1, 0, 2))

    def drpair(wmat, ncol):
        # [C, ncol] -> [128, NP3, 2, ncol]: DR pairs of feature chunks
        r = wmat.reshape(NP3, 2, P, ncol)
        return np.ascontiguousarray(r.transpose(2, 0, 1, 3).astype(e4))

    def hilo(wmat):
        # [Cin, ncol] -> (hi, 16*lo) fp8 stack [Cin, 2, ncol]; the rhs lo
        # slice carries the matching 1/16 so the product reassembles W*x.
        hi = wmat.astype(e4)
        lo = ((wmat - hi.astype(np.float32)) * 16.0).astype(e4)
        return np.stack([hi, lo], axis=1)

    # K/Q: per head-group matmul columns [h0 f0-31 | h1 | h2 | h3] per half
    def qk_tile(wmat):
        out = np.zeros((3, P, 2, NP3, 2, P), np.float32)
        wsum = np.zeros((3, 2, P), np.float32)
        for hg in range(3):
            cols = []
            for hf in range(2):
                for hi in range(4):
                    h_ = 4 * hg + hi
                    cols.append(wmat[:, h_ * HD + hf * 32: h_ * HD + hf * 32 + 32])
            m = np.concatenate(cols, axis=1)               # [C, 256]
            for hf in range(2):
                blk = m[:, hf * P:(hf + 1) * P]            # [C, 128]
                out[hg, :, hf] = blk.reshape(NP3, 2, P, P).transpose(2, 0, 1, 3)
                wsum[hg, hf] = blk.sum(axis=0) / WS
        return out.astype(e4), wsum

    wq_t, wsq = qk_tile(Wq)
    wk_t, wsk = qk_tile(Wk)
    Wqk_t = np.stack([wq_t, wk_t])
    wsqk = np.zeros((1, 2, 3, 2, 2, P), np.float32)
    wsqk[0, 0, :, :, 0, :] = wsq
    wsqk[0, 1, :, :, 0, :] = wsk

    Wv_t = np.stack([drpair(Wv[:, 0:384], 384), drpair(Wv[:, 384:768], 384)])
    wsv = np.zeros((1, 2, 2, 384), np.float32)
    wsv[0, :, 0, :] = (Wv.sum(axis=0) / WS).reshape(2, 384)

    Wap_t = drpair(Wap, C)
    # Wfc hi/lo: [C, 2, 3072] -> [P, 12 gp, NF fchunk, 2, 256]
    fc_hl = hilo(Wfc)                                      # [C, 2, 3072]
    fc_hl = fc_hl.reshape(NF, P, 2, 12, 256)               # (f, p, sl, gp, col)
    Wfc_t = np.ascontiguousarray(fc_hl.transpose(1, 3, 0, 2, 4))
    # Wmp: plain fp8, DR pairs of h-chunks -> [P, NF mo, NH2 pair, 2, 128]
    mp = Wmp.astype(e4).reshape(NH2, 2, P, NF, P)          # (pair, sl, p, mo, col)
    Wmp_t = np.ascontiguousarray(mp.transpose(2, 3, 0, 1, 4))

    ident = np.zeros((P, 2, P), np.float32)
    ident[:, 0, :] = np.eye(P)

    full = {
        "Wqk": Wqk_t, "Wv": Wv_t, "Wap": Wap_t, "Wfc": Wfc_t, "Wmp": Wmp_t,
        "wsqk": wsqk.astype(e4), "wsv": wsv.astype(e4), "ident": ident.astype(e4),
    }
    unit_gb = bool(
        np.all(np.asarray(inputs["ln2_g"]) == 1.0)
        and np.all(np.asarray(inputs["ln2_b"]) == 0.0)
        and np.all(g1 == 1.0))
    if not unit_gb:
        g2 = np.asarray(inputs["ln2_g"], np.float32)
        b2 = np.asarray(inputs["ln2_b"], np.float32)
        full["g2c"] = np.ascontiguousarray(g2.reshape(NF, P).T)
        full["b2c"] = np.ascontiguousarray(b2.reshape(NF, P).T)

    in_maps = []
    for c in range(8):
        b_, p_ = c // 4, c % 4
        blocks = [bs - 1 - p_ for bs in SLOT_BOUNDS]
        xb_ = x[b_]
        own = np.concatenate([np.arange(bk * P, (bk + 1) * P) for bk in blocks])
        xcat = np.concatenate([xb_, xb_[own]], axis=0).T   # [C, 2560]
        mask = np.zeros((P, 4, 4, 2, P), np.float32)
        kp = np.arange(P)[None, :]
        qv = np.arange(P)[:, None]
        for s, BS in enumerate(SLOT_BOUNDS):
            E = BS - p_
            blk = BS - 1 - p_
            for j in range(4):
                kc = BS - 4 + j
                if kc >= E:
                    mask[:, s, j, 0, :] = MASK_NEG
                elif kc == E - 1:
                    mask[:, s, j, 0, :] = np.where(
                        kc * P + kp <= blk * P + qv, 0.0, MASK_NEG)
        m = dict(full)
        m["x8"] = fchunk(xcat, NTILE * 512).astype(e4)
        m["xo"] = fchunk(np.ascontiguousarray(xb_[own].T), OWN)
        m["maskT"] = mask.astype(e4)
        in_maps.append(m)
    return in_maps


def assemble_output(results):
    out = np.empty((B, T, C), dtype=np.float32)
    for c in range(8):
        b_, p_ = c // 4, c % 4
        oT = results[c]["outT"].T
        for s, BS in enumerate(SLOT_BOUNDS):
            blk = BS - 1 - p_
            out[b_, blk * P:(blk + 1) * P, :] = oT[s * P:(s + 1) * P, :]
    return out


_CACHED_NC = {}


def kernel(**inputs):
    from concourse.bass_utils import run_bass_kernel_spmd

    unit_gb = bool(
        np.all(np.asarray(inputs["ln1_g"]) == 1.0)
        and np.all(np.asarray(inputs["ln2_g"]) == 1.0)
        and np.all(np.asarray(inputs["ln1_b"]) == 0.0)
        and np.all(np.asarray(inputs["ln2_b"]) == 0.0))
    if unit_gb not in _CACHED_NC:
        _CACHED_NC[unit_gb] = build_program(unit_gb=unit_gb)
    in_maps = make_core_inputs(inputs)
    res = run_bass_kernel_spmd(_CACHED_NC[unit_gb], in_maps,
                               core_ids=list(range(8)))
    return assemble_output(res.results)


if __name__ == "__main__":
    nc = build_program()
    print("program built ok")

